# revision 3
# baseline (speedup 1.0000x reference)
"""Trainium2 Bass kernel for nn_Disp_61125974557155.

Computes: trilinear upsample of a cost volume [B,1,48,64,128] ->
[B,193,256,512] (align_corners=False, edge-replicated), softmin over
disparity, disparity regression -> [B,256,512].

Design (per core; 8 cores = 2 batches x 4 H'-quarters):
  - Host: edge-pad x (replicate), slice the core's H-halo shard, and stack a
    copy shifted by one h-row on partitions 50..99 (sharding/memory movement
    only, no arithmetic).
  - DVE: W-axis 4x lerp at low resolution -> xsw [100, 17, 4, 128] bf16.
  - PE: D-expansion with the H-axis lerp folded in (dup-shifted 100-row
    operand), all-bf16.  Tiles are paired (j1=2m, j2=2m+1 share the same
    xsw row): window = 3 PSUM banks [A1 | B1+B2 | A2] = [128, 1536].  The
    two 64-row B-chunks share the middle bank via a pair of accumulating
    matmuls whose stationaries are zero-padded into disjoint column halves
    -- no pad columns reach the ACT engine.
  - ACT: one exp per window (1536 cols) -> bf16 e-window.  ACT is the
    pacing engine; everything else hides under it.  The first and last
    windows are exp'd per-bank so ACT starts earlier and the final stats
    start earlier.
  - PE: flip stat matmuls (e-slice stationary [K,128], rmat moving)
    accumulate (S0, S1) into one persistent pixel-major PSUM bank; the
    packed B-bank yields both tiles' stats in one matmul (4 moving cols).
  - DVE: out = S1 * recip(S0), shipped pixel-major; the host does the final
    (j, w') reindex while unsharding (memory movement only).
"""

import numpy as np
from contextlib import ExitStack

import concourse.bass as bass
import concourse.bacc as bacc
import concourse.tile as tile
from concourse import mybir
from concourse.bass_utils import run_bass_kernel_spmd
from concourse.tile_rust import add_dep_helper

F32 = mybir.dt.float32
BF16 = mybir.dt.bfloat16

MAXDISP = 192
DP = MAXDISP + 1      # 193 disparities
ND = 192              # distinct d' rows (d'=0,1 coincide)
KD = 48               # low-res D
KP = KD + 2           # padded k' (edge-replicated)
NCORES = 8
WH = (0.625, 0.875, 0.125, 0.375)   # H lerp fracs per r = h' % 4
NROW = 17                            # h-rows in dup-packed shard (l = 0..16)
ROW_GROUPS = ((0, 1), (1, 1), (2, 2), (4, 4), (8, 4), (12, 4), (16, 1))
NPAIR = 32                           # tile pairs (j1=2m, j2=2m+1)


def _build_ad() -> np.ndarray:
    """A_D [192, 50]: head-dedup'd D-axis upsample matrix on padded k'.

    Row 0 covers d' in {0,1} (edge-replicated pair, canonicalized to weight
    1.0 on xp[1]); row i (i>=1) covers d' = i+1."""
    full = np.zeros((DP, KP), dtype=np.float64)
    for dp in range(DP):
        i = (dp + 0.5) * KD / DP - 0.5
        fl = int(np.floor(i))
        fr = i - fl
        full[dp, fl + 1] += 1.0 - fr
        full[dp, fl + 2] += fr
    for dp in (0, 1):
        assert abs(full[dp, 0] + full[dp, 1] - 1.0) < 1e-12 and full[dp, 2:].sum() == 0
        full[dp, 0], full[dp, 1] = 0.0, 1.0
    return full[1:]


def _build_consts():
    ad = _build_ad()                                       # [192, 50]
    ad_a = ad[0:128]                                       # d' {0,1},2..128
    ad_b = ad[128:192]                                     # d' 129..192
    # amat [100, 8, 128], slab order = first-use order: the even-pair
    # stationaries [A0, Blo0, Bhi1, A1] then the odd-pair ones
    # [A2, Blo2, Bhi3, A3].  B slabs are zero-padded into the column half
    # their tile's B-rows occupy in the shared middle PSUM bank, so the pair
    # of B matmuls accumulates [B1; B2] with full-partition writes (and
    # FWL-friendly 128-col loads).
    amat = np.zeros((2 * KP, 8, 128), dtype=np.float64)

    def fill(slab, r, mat, h0):
        amat[:KP, slab, h0 : h0 + mat.shape[0]] = (1.0 - WH[r]) * mat.T
        amat[KP:, slab, h0 : h0 + mat.shape[0]] = WH[r] * mat.T

    for half, (r1, r2) in enumerate(((0, 1), (2, 3))):
        fill(4 * half + 0, r1, ad_a, 0)
        fill(4 * half + 1, r1, ad_b, 0)
        fill(4 * half + 2, r2, ad_b, 64)
        fill(4 * half + 3, r2, ad_a, 0)
    # rmat [128, 6]: cols 0:2 = A-chunk (S0w, S1w) for d-rows 0:128 (row0 is
    # the {0,1} pair: weights 2 and 0+1); cols 2:4 = B-stats of the pair's
    # FIRST tile (nonzero on partitions 0:64 = d' 129..192); cols 4:6 = the
    # SECOND tile's (partitions 64:128).  All entries are integers <= 256 so
    # exactly representable in bf16.
    s0w = np.ones(ND)
    s1w = np.arange(1, ND + 1, dtype=np.float64)
    s0w[0], s1w[0] = 2.0, 1.0            # row0 = {0,1}: S0w 2, S1w 0+1
    rmat = np.zeros((128, 6), dtype=np.float64)
    rmat[:, 0] = s0w[0:128]
    rmat[:, 1] = s1w[0:128]
    rmat[0:64, 2] = s0w[128:192]
    rmat[0:64, 3] = s1w[128:192]
    rmat[64:128, 4] = s0w[128:192]
    rmat[64:128, 5] = s1w[128:192]
    rbf = rmat.astype(np.float32).astype(mybir.dt.np(BF16))
    assert np.array_equal(rbf.astype(np.float64), rmat)
    return (
        np.ascontiguousarray(amat.reshape(2 * KP, 8 * 128), dtype=np.float32)
        .astype(mybir.dt.np(BF16)),
        rbf,
    )


def _build_nc() -> bass.Bass:
    nc = bacc.Bacc()
    xsd = nc.declare_dram_parameter("xsd", [2 * KP, NROW * 130], F32, isOutput=False)
    amat = nc.declare_dram_parameter("amat", [2 * KP, 8 * 128], BF16, isOutput=False)
    rmat = nc.declare_dram_parameter("rmat", [128, 6], BF16, isOutput=False)
    outp = nc.declare_dram_parameter("out", [128, 256], F32, isOutput=True)

    xsd_v = xsd.rearrange("p (h w) -> p h w", h=NROW)
    amat_v = amat.rearrange("p (v d) -> p v d", v=8)

    mult = mybir.AluOpType.mult
    add = mybir.AluOpType.add
    exp_fn = mybir.ActivationFunctionType.Exp

    # Chain PE matmuls in emission order so the scheduler keeps them
    # back-to-back (p-state / HAM ramp) and same-dtype runs contiguous.
    last_pe = [None]

    def pe_matmul(*args, **kwargs):
        ins = nc.tensor.matmul(*args, **kwargs)
        if last_pe[0] is not None:
            add_dep_helper(ins.ins, last_pe[0].ins, False,
                           "keep matmul bursts contiguous")
        last_pe[0] = ins
        return ins

    with ExitStack() as ctx:
        tc = ctx.enter_context(tile.TileContext(nc))
        singles = ctx.enter_context(tc.tile_pool(name="singles", bufs=1))
        tmp_pool = ctx.enter_context(tc.tile_pool(name="tmp", bufs=4))
        epool = ctx.enter_context(tc.tile_pool(name="epool", bufs=4))
        fin = ctx.enter_context(tc.tile_pool(name="fin", bufs=1))
        pvol = ctx.enter_context(tc.tile_pool(name="pvol", bufs=2, space="PSUM"))
        pstat = ctx.enter_context(tc.tile_pool(name="pstat", bufs=1, space="PSUM"))

        # ---- input loads ----
        # xsd row groups go on the sync HWDGE queue (lands earlier than the
        # gpsimd SWDGE path here), smallest/first-needed group first; the
        # stationaries go on gpsimd in two first-use-ordered chunks so both
        # queues run in parallel and nothing serializes behind 10 tiny DMAs.
        s_xsd = []
        for g, (g0, gn) in enumerate(ROW_GROUPS):
            t_x = singles.tile([2 * KP, gn, 130], F32, tag=f"xsd{g}")
            nc.sync.dma_start(out=t_x, in_=xsd_v[:, g0 : g0 + gn, :])
            s_xsd.append(t_x)
        s_am = {}
        amkeys = ((("A", 0), ("B", 0), ("B", 1), ("A", 1)),
                  (("A", 2), ("B", 2), ("B", 3), ("A", 3)))
        for half in range(2):
            t_a = singles.tile([2 * KP, 4, 128], BF16, tag=f"am{half}")
            nc.gpsimd.dma_start(
                out=t_a, in_=amat_v[:, 4 * half : 4 * half + 4, :]
            )
            for v, key in enumerate(amkeys[half]):
                s_am[key] = t_a[:, v, :]
        s_rm = singles.tile([128, 6], BF16, tag="rm")
        nc.gpsimd.dma_start(out=s_rm, in_=rmat[:, :])

        # ---- W-axis 4x lerp at low res, rw-major planes (bf16 out) ----
        s_xsw = []
        for g, (g0, gn) in enumerate(ROW_GROUPS):
            t_w = singles.tile([2 * KP, gn, 4, 128], BF16, tag=f"xsw{g}")
            t_d = tmp_pool.tile([2 * KP, gn, 129], F32, tag="wld")
            nc.vector.tensor_sub(
                t_d, s_xsd[g][:, :, 0:129], s_xsd[g][:, :, 1:130]
            )
            for rw, (coef, dc, hc) in enumerate(
                ((0.375, 0, 1), (0.125, 0, 1), (0.875, 1, 2), (0.625, 1, 2))
            ):
                nc.vector.scalar_tensor_tensor(
                    out=t_w[:, :, rw, :],
                    in0=t_d[:, :, dc : dc + 128],
                    scalar=coef,
                    in1=s_xsd[g][:, :, hc : hc + 128],
                    op0=mult,
                    op1=add,
                )
            s_xsw.append(t_w)

        def xsw_row(l: int) -> bass.AP:
            for g, (g0, gn) in enumerate(ROW_GROUPS):
                if g0 <= l < g0 + gn:
                    return s_xsw[g][:, l - g0, :, :]
            raise IndexError(l)

        # ---- persistent pixel-major stats bank ----
        # ps[p, q, j, :] = (S0, S1) of output pixel (h'-row j, w' = 4*p + q)
        ps = pstat.tile([128, 512], F32, tag="ps")
        ps_v = ps.rearrange("p (q j s) -> p q j s", q=4, s=2)
        ps_v4 = ps.rearrange("p (q m s4) -> p q m s4", q=4, s4=4)

        # ---- main loop: 32 tile pairs, one 3-bank window each ----
        # Pair m: j1 = 2m (r1 in {0,2}), j2 = 2m+1 (r2 = r1+1); both share
        # xsw row l = m//2 + m%2.
        ewins = {}

        def pair_info(m):
            t, odd = divmod(m, 2)
            r1 = 2 * odd
            return t + odd, r1, r1 + 1

        def emit_slots(m, pv):
            l, r1, r2 = pair_info(m)
            rhs = xsw_row(l).rearrange("p q s -> p (q s)")   # [100, 512]
            pe_matmul(pv[:, 0:512], s_am[("A", r1)], rhs,
                      start=True, stop=True)
            # B1 (cols 0:64 live) then B2 (cols 64:128 live) accumulate into
            # the shared middle bank; the zero column-halves keep every write
            # full-partition.
            pe_matmul(pv[:, 512:1024], s_am[("B", r1)], rhs,
                      start=True, stop=False, skip_group_check=True)
            pe_matmul(pv[:, 512:1024], s_am[("B", r2)], rhs,
                      start=False, stop=True, skip_group_check=True)
            pe_matmul(pv[:, 1024:1536], s_am[("A", r2)], rhs,
                      start=True, stop=True)

        def emit_stats(m, first, banks=(0, 1, 2)):
            et = ewins[m]
            j1 = 2 * m
            for q in range(4):
                if 0 in banks:
                    pe_matmul(
                        ps_v[:, q, j1, :],
                        et[:, 128 * q : 128 * q + 128],
                        s_rm[:, 0:2],
                        start=(first and q == 0), stop=False,
                        skip_group_check=True,
                    )
                if 2 in banks:
                    pe_matmul(
                        ps_v[:, q, j1 + 1, :],
                        et[:, 1024 + 128 * q : 1024 + 128 * q + 128],
                        s_rm[:, 0:2],
                        start=False,
                        stop=(m == NPAIR - 1 and q == 3),
                        skip_group_check=True,
                    )
                if 1 in banks:
                    pe_matmul(
                        ps_v4[:, q, m, :],
                        et[:, 512 + 128 * q : 512 + 128 * q + 128],
                        s_rm[:, 2:6],
                        start=False, stop=False,
                        skip_group_check=True,
                    )

        for m in range(NPAIR):
            pv = pvol.tile([128, 1536], F32, tag="pv")
            et = epool.tile([128, 1536], BF16, tag="e")
            ewins[m] = et
            emit_slots(m, pv)
            if m == 0:
                # Per-bank exps: ACT starts right after the first slot
                # matmul instead of after all four.
                for b in range(3):
                    nc.scalar.activation(
                        et[:, 512 * b : 512 * b + 512],
                        pv[:, 512 * b : 512 * b + 512],
                        exp_fn, scale=-1.0,
                    )
            elif m == NPAIR - 1:
                # Per-bank exps + interleaved stats: the final stats don't
                # all wait for the full last window.
                for b in range(3):
                    nc.scalar.activation(
                        et[:, 512 * b : 512 * b + 512],
                        pv[:, 512 * b : 512 * b + 512],
                        exp_fn, scale=-1.0,
                    )
                    emit_stats(m, False, banks=(b,))
            else:
                nc.scalar.activation(et, pv, exp_fn, scale=-1.0)
            # emit stats lagging two windows: exp(m-2) is complete by the
            # time slots(m) could start (its pvol buffer was recycled), so
            # these never stall the PE queue.
            if 2 <= m < NPAIR - 1:
                emit_stats(m - 2, m == 2)
        emit_stats(NPAIR - 3, False)
        emit_stats(NPAIR - 2, False)

        # ---- finalize: out = S1 * recip(S0), pixel-major [p, q, j] ----
        rec = fin.tile([128, 4, 64], F32, tag="rec")
        oo = fin.tile([128, 4, 64], F32, tag="oo")
        nc.vector.reciprocal_approx_fast(rec, ps_v[:, :, :, 0])
        nc.vector.tensor_mul(oo, ps_v[:, :, :, 1], rec)
        nc.sync.dma_start(out=outp[:, :], in_=oo.rearrange("p q j -> p (q j)"))

    nc.compile()
    return nc


_CACHE: dict = {}


def _shard_inputs(x: np.ndarray):
    """Edge-pad and slice per-core shards (memory movement only)."""
    xpad = np.pad(x[:, 0], ((0, 0), (1, 1), (1, 3), (1, 1)), mode="edge")
    amat, rmat = _build_consts()
    in_maps = []
    for c in range(NCORES):
        b, q = divmod(c, 4)
        xs = xpad[b][:, 16 * q : 16 * q + 18, :]          # [50, 18, 130]
        xsd = np.concatenate([xs[:, 0:17, :], xs[:, 1:18, :]], axis=0)
        xsd = np.ascontiguousarray(
            xsd.reshape(2 * KP, NROW * 130), dtype=np.float32
        )
        in_maps.append({"xsd": xsd, "amat": amat, "rmat": rmat})
    return in_maps


def kernel(x: np.ndarray, _trace: bool = False, _tmpdir=None):
    x = np.asarray(x, dtype=np.float32)
    assert x.shape == (2, 1, 48, 64, 128), x.shape
    if "nc" not in _CACHE:
        _CACHE["nc"] = _build_nc()
    nc = _CACHE["nc"]
    in_maps = _shard_inputs(x)
    res = run_bass_kernel_spmd(
        nc, in_maps, list(range(NCORES)), trace=_trace, tmpdir=_tmpdir
    )
    out = np.zeros((2, 256, 512), dtype=np.float32)
    for c in range(NCORES):
        b, q = divmod(c, 4)
        # core output is pixel-major [p, q, j]: reindex to [j, w' = 4p+q]
        # (host-side memory movement only).
        oo = res.results[c]["out"].reshape(128, 4, 64)
        out[b, 64 * q : 64 * (q + 1), :] = (
            oo.transpose(2, 0, 1).reshape(64, 512)
        )
    if _trace:
        return out, res
    return out


# revision 6
# speedup vs baseline: 1.4821x; 1.4821x over previous
"""Trainium2 Bass kernel for nn_Disp_61125974557155.

Computes: trilinear upsample of a cost volume [B,1,48,64,128] ->
[B,193,256,512] (align_corners=False, edge-replicated), softmin over
disparity, disparity regression -> [B,256,512].

Design (per core; 8 cores = 2 batches x 4 H'-quarters):
  - Host: edge-pad x (replicate), slice the core's H-halo shard, and stack a
    copy shifted by one h-row on partitions 50..99 (sharding/memory movement
    only, no arithmetic).
  - DVE: W-axis 4x lerp at low resolution -> xsw [100, 17, 4, 128] bf16.
  - PE: D-expansion with the H-axis lerp folded in (dup-shifted 100-row
    operand), all-bf16.  Tiles are paired (j1=2m, j2=2m+1 share the same
    xsw row): window = 3 PSUM banks [A1 | B1+B2 | A2] = [128, 1536].  The
    two 64-row B-chunks share the middle bank via a pair of accumulating
    matmuls whose stationaries are zero-padded into disjoint column halves
    -- no pad columns reach the ACT engine.
  - ACT: one exp per window (1536 cols) -> bf16 e-window.  ACT is the
    pacing engine; everything else hides under it.  The first and last
    windows are exp'd per-bank so ACT starts earlier and the final stats
    start earlier.
  - PE: flip stat matmuls (e-slice stationary [K,128], rmat moving)
    accumulate (S0, S1) into one persistent pixel-major PSUM bank; the
    packed B-bank yields both tiles' stats in one matmul (4 moving cols).
  - DVE: out = S1 * recip(S0), shipped pixel-major; the host does the final
    (j, w') reindex while unsharding (memory movement only).
"""

import numpy as np
from contextlib import ExitStack

import concourse.bass as bass
import concourse.bacc as bacc
import concourse.tile as tile
from concourse import mybir
from concourse.bass_utils import run_bass_kernel_spmd
from concourse.tile_rust import add_dep_helper

F32 = mybir.dt.float32
BF16 = mybir.dt.bfloat16

MAXDISP = 192
DP = MAXDISP + 1      # 193 disparities
ND = 192              # distinct d' rows (d'=0,1 coincide)
KD = 48               # low-res D
KP = KD + 2           # padded k' (edge-replicated)
NCORES = 8
WH = (0.625, 0.875, 0.125, 0.375)   # H lerp fracs per r = h' % 4
NROW = 17                            # h-rows in dup-packed shard (l = 0..16)
ROW_GROUPS = ((0, 1), (1, 1), (2, 2), (4, 4), (8, 4), (12, 4), (16, 1))
NPAIR = 32                           # tile pairs (j1=2m, j2=2m+1)


def _build_ad() -> np.ndarray:
    """A_D [192, 50]: head-dedup'd D-axis upsample matrix on padded k'.

    Row 0 covers d' in {0,1} (edge-replicated pair, canonicalized to weight
    1.0 on xp[1]); row i (i>=1) covers d' = i+1."""
    full = np.zeros((DP, KP), dtype=np.float64)
    for dp in range(DP):
        i = (dp + 0.5) * KD / DP - 0.5
        fl = int(np.floor(i))
        fr = i - fl
        full[dp, fl + 1] += 1.0 - fr
        full[dp, fl + 2] += fr
    for dp in (0, 1):
        assert abs(full[dp, 0] + full[dp, 1] - 1.0) < 1e-12 and full[dp, 2:].sum() == 0
        full[dp, 0], full[dp, 1] = 0.0, 1.0
    return full[1:]


def _build_consts():
    ad = _build_ad()                                       # [192, 50]
    ad_a = ad[0:128]                                       # d' {0,1},2..128
    ad_b = ad[128:192]                                     # d' 129..192
    # amat [100, 8, 128], slab order = first-use order: the even-pair
    # stationaries [A0, Blo0, Bhi1, A1] then the odd-pair ones
    # [A2, Blo2, Bhi3, A3].  B slabs are zero-padded into the column half
    # their tile's B-rows occupy in the shared middle PSUM bank, so the pair
    # of B matmuls accumulates [B1; B2] with full-partition writes (and
    # FWL-friendly 128-col loads).
    amat = np.zeros((2 * KP, 8, 128), dtype=np.float64)

    def fill(slab, r, mat, h0):
        amat[:KP, slab, h0 : h0 + mat.shape[0]] = (1.0 - WH[r]) * mat.T
        amat[KP:, slab, h0 : h0 + mat.shape[0]] = WH[r] * mat.T

    for half, (r1, r2) in enumerate(((0, 1), (2, 3))):
        fill(4 * half + 0, r1, ad_a, 0)
        fill(4 * half + 1, r1, ad_b, 0)
        fill(4 * half + 2, r2, ad_b, 64)
        fill(4 * half + 3, r2, ad_a, 0)
    # rmat [128, 6]: cols 0:2 = A-chunk (S0w, S1w) for d-rows 0:128 (row0 is
    # the {0,1} pair: weights 2 and 0+1); cols 2:4 = B-stats of the pair's
    # FIRST tile (nonzero on partitions 0:64 = d' 129..192); cols 4:6 = the
    # SECOND tile's (partitions 64:128).  All entries are integers <= 256 so
    # exactly representable in bf16.
    s0w = np.ones(ND)
    s1w = np.arange(1, ND + 1, dtype=np.float64)
    s0w[0], s1w[0] = 2.0, 1.0            # row0 = {0,1}: S0w 2, S1w 0+1
    rmat = np.zeros((128, 6), dtype=np.float64)
    rmat[:, 0] = s0w[0:128]
    rmat[:, 1] = s1w[0:128]
    rmat[0:64, 2] = s0w[128:192]
    rmat[0:64, 3] = s1w[128:192]
    rmat[64:128, 4] = s0w[128:192]
    rmat[64:128, 5] = s1w[128:192]
    rbf = rmat.astype(np.float32).astype(mybir.dt.np(BF16))
    assert np.array_equal(rbf.astype(np.float64), rmat)
    return (
        np.ascontiguousarray(amat.reshape(2 * KP, 8 * 128), dtype=np.float32)
        .astype(mybir.dt.np(BF16)),
        rbf,
    )


def _build_nc() -> bass.Bass:
    nc = bacc.Bacc()
    xsd = nc.declare_dram_parameter("xsd", [2 * KP, NROW * 130], F32, isOutput=False)
    amat = nc.declare_dram_parameter("amat", [2 * KP, 8 * 128], BF16, isOutput=False)
    rmat = nc.declare_dram_parameter("rmat", [128, 6], BF16, isOutput=False)
    outp = nc.declare_dram_parameter("out", [128, 256], F32, isOutput=True)

    xsd_v = xsd.rearrange("p (h w) -> p h w", h=NROW)
    amat_v = amat.rearrange("p (v d) -> p v d", v=8)

    mult = mybir.AluOpType.mult
    add = mybir.AluOpType.add
    exp_fn = mybir.ActivationFunctionType.Exp

    # Chain PE matmuls in emission order so the scheduler keeps them
    # back-to-back (p-state / HAM ramp) and same-dtype runs contiguous.
    last_pe = [None]

    def pe_matmul(*args, **kwargs):
        ins = nc.tensor.matmul(*args, **kwargs)
        if last_pe[0] is not None:
            add_dep_helper(ins.ins, last_pe[0].ins, False,
                           "keep matmul bursts contiguous")
        last_pe[0] = ins
        return ins

    with ExitStack() as ctx:
        tc = ctx.enter_context(tile.TileContext(nc))
        singles = ctx.enter_context(tc.tile_pool(name="singles", bufs=1))
        tmp_pool = ctx.enter_context(tc.tile_pool(name="tmp", bufs=4))
        epool = ctx.enter_context(tc.tile_pool(name="epool", bufs=4))
        fin = ctx.enter_context(tc.tile_pool(name="fin", bufs=1))
        pvol = ctx.enter_context(tc.tile_pool(name="pvol", bufs=2, space="PSUM"))
        pstat = ctx.enter_context(tc.tile_pool(name="pstat", bufs=1, space="PSUM"))
        pwup = ctx.enter_context(tc.tile_pool(name="pwup", bufs=1, space="PSUM"))

        # ---- PE warm-up ----
        # The HAM clock gate only lifts the PE to 2.4 GHz after ~3.4us of
        # sustained activity, and a cold (1.2 GHz) PE cannot keep up with the
        # ACT-paced main loop -- the resulting dependency stalls then keep it
        # cold forever.  Burn the otherwise-idle DMA head on a burst of dummy
        # matmuls (zeros into the spare 8th PSUM bank) so the PE enters the
        # loop already warm.
        s_wup = singles.tile([128, 640], BF16, tag="wup")
        nc.gpsimd.memset(s_wup, 0.0)

        # ---- input loads ----
        # xsd row groups go on the sync HWDGE queue (lands earlier than the
        # gpsimd SWDGE path here), smallest/first-needed group first; the
        # stationaries go on gpsimd in two first-use-ordered chunks so both
        # queues run in parallel and nothing serializes behind 10 tiny DMAs.
        s_xsd = []
        for g, (g0, gn) in enumerate(ROW_GROUPS):
            t_x = singles.tile([2 * KP, gn, 130], F32, tag=f"xsd{g}")
            nc.sync.dma_start(out=t_x, in_=xsd_v[:, g0 : g0 + gn, :])
            s_xsd.append(t_x)
        s_am = {}
        amkeys = ((("A", 0), ("B", 0), ("B", 1), ("A", 1)),
                  (("A", 2), ("B", 2), ("B", 3), ("A", 3)))
        for half in range(2):
            t_a = singles.tile([2 * KP, 4, 128], BF16, tag=f"am{half}")
            nc.gpsimd.dma_start(
                out=t_a, in_=amat_v[:, 4 * half : 4 * half + 4, :]
            )
            for v, key in enumerate(amkeys[half]):
                s_am[key] = t_a[:, v, :]
        s_rm = singles.tile([128, 6], BF16, tag="rm")
        nc.gpsimd.dma_start(out=s_rm, in_=rmat[:, :])

        wb = pwup.tile([128, 512], F32, tag="wb")
        for _ in range(12):
            pe_matmul(wb, s_wup[:, 0:128], s_wup[:, 128:640],
                      start=True, stop=True, skip_group_check=True)

        # ---- W-axis 4x lerp at low res, rw-major planes (bf16 out) ----
        s_xsw = []
        for g, (g0, gn) in enumerate(ROW_GROUPS):
            t_w = singles.tile([2 * KP, gn, 4, 128], BF16, tag=f"xsw{g}")
            t_d = tmp_pool.tile([2 * KP, gn, 129], F32, tag="wld")
            nc.vector.tensor_sub(
                t_d, s_xsd[g][:, :, 0:129], s_xsd[g][:, :, 1:130]
            )
            for rw, (coef, dc, hc) in enumerate(
                ((0.375, 0, 1), (0.125, 0, 1), (0.875, 1, 2), (0.625, 1, 2))
            ):
                nc.vector.scalar_tensor_tensor(
                    out=t_w[:, :, rw, :],
                    in0=t_d[:, :, dc : dc + 128],
                    scalar=coef,
                    in1=s_xsd[g][:, :, hc : hc + 128],
                    op0=mult,
                    op1=add,
                )
            s_xsw.append(t_w)

        def xsw_row(l: int) -> bass.AP:
            for g, (g0, gn) in enumerate(ROW_GROUPS):
                if g0 <= l < g0 + gn:
                    return s_xsw[g][:, l - g0, :, :]
            raise IndexError(l)

        # ---- persistent pixel-major stats bank ----
        # ps[p, q, j, :] = (S0, S1) of output pixel (h'-row j, w' = 4*p + q)
        ps = pstat.tile([128, 512], F32, tag="ps")
        ps_v = ps.rearrange("p (q j s) -> p q j s", q=4, s=2)
        ps_v4 = ps.rearrange("p (q m s4) -> p q m s4", q=4, s4=4)

        # ---- main loop: 32 tile pairs, one 3-bank window each ----
        # Pair m: j1 = 2m (r1 in {0,2}), j2 = 2m+1 (r2 = r1+1); both share
        # xsw row l = m//2 + m%2.
        ewins = {}

        def pair_info(m):
            t, odd = divmod(m, 2)
            r1 = 2 * odd
            return t + odd, r1, r1 + 1

        def emit_slots(m, pv):
            l, r1, r2 = pair_info(m)
            rhs = xsw_row(l).rearrange("p q s -> p (q s)")   # [100, 512]
            pe_matmul(pv[:, 0:512], s_am[("A", r1)], rhs,
                      start=True, stop=True)
            # B1 (cols 0:64 live) then B2 (cols 64:128 live) accumulate into
            # the shared middle bank; the zero column-halves keep every write
            # full-partition.
            pe_matmul(pv[:, 512:1024], s_am[("B", r1)], rhs,
                      start=True, stop=False, skip_group_check=True)
            pe_matmul(pv[:, 512:1024], s_am[("B", r2)], rhs,
                      start=False, stop=True, skip_group_check=True)
            pe_matmul(pv[:, 1024:1536], s_am[("A", r2)], rhs,
                      start=True, stop=True)

        def emit_stats(m, first, banks=(0, 1, 2)):
            et = ewins[m]
            j1 = 2 * m
            for q in range(4):
                if 0 in banks:
                    pe_matmul(
                        ps_v[:, q, j1, :],
                        et[:, 128 * q : 128 * q + 128],
                        s_rm[:, 0:2],
                        start=(first and q == 0), stop=False,
                        skip_group_check=True,
                    )
                if 2 in banks:
                    pe_matmul(
                        ps_v[:, q, j1 + 1, :],
                        et[:, 1024 + 128 * q : 1024 + 128 * q + 128],
                        s_rm[:, 0:2],
                        start=False,
                        stop=(m == NPAIR - 1 and q == 3),
                        skip_group_check=True,
                    )
                if 1 in banks:
                    pe_matmul(
                        ps_v4[:, q, m, :],
                        et[:, 512 + 128 * q : 512 + 128 * q + 128],
                        s_rm[:, 2:6],
                        start=False, stop=False,
                        skip_group_check=True,
                    )

        for m in range(NPAIR):
            pv = pvol.tile([128, 1536], F32, tag="pv")
            et = epool.tile([128, 1536], BF16, tag="e")
            ewins[m] = et
            # stats lag two windows: exp(m-2) is complete by the time
            # slots(m) could start (its pvol buffer was recycled).  Emitting
            # them BEFORE slots(m) lets them fill the PE queue while
            # slots(m) still waits on that buffer recycle.
            if m >= 2:
                emit_stats(m - 2, m == 2)
            emit_slots(m, pv)
            if m == NPAIR - 1:
                # Per-bank exps + interleaved stats: the final stats don't
                # all wait for the full last window.
                for b in range(3):
                    nc.scalar.activation(
                        et[:, 512 * b : 512 * b + 512],
                        pv[:, 512 * b : 512 * b + 512],
                        exp_fn, scale=-1.0,
                    )
                    emit_stats(m, False, banks=(b,))
            else:
                nc.scalar.activation(et, pv, exp_fn, scale=-1.0)
        emit_stats(NPAIR - 2, False)

        # ---- finalize: out = S1 * recip(S0), pixel-major [p, q, j] ----
        rec = fin.tile([128, 4, 64], F32, tag="rec")
        oo = fin.tile([128, 4, 64], F32, tag="oo")
        nc.vector.reciprocal_approx_fast(rec, ps_v[:, :, :, 0])
        nc.vector.tensor_mul(oo, ps_v[:, :, :, 1], rec)
        nc.sync.dma_start(out=outp[:, :], in_=oo.rearrange("p q j -> p (q j)"))

    nc.compile()
    return nc


_CACHE: dict = {}


def _shard_inputs(x: np.ndarray):
    """Edge-pad and slice per-core shards (memory movement only)."""
    xpad = np.pad(x[:, 0], ((0, 0), (1, 1), (1, 3), (1, 1)), mode="edge")
    amat, rmat = _build_consts()
    in_maps = []
    for c in range(NCORES):
        b, q = divmod(c, 4)
        xs = xpad[b][:, 16 * q : 16 * q + 18, :]          # [50, 18, 130]
        xsd = np.concatenate([xs[:, 0:17, :], xs[:, 1:18, :]], axis=0)
        xsd = np.ascontiguousarray(
            xsd.reshape(2 * KP, NROW * 130), dtype=np.float32
        )
        in_maps.append({"xsd": xsd, "amat": amat, "rmat": rmat})
    return in_maps


def kernel(x: np.ndarray, _trace: bool = False, _tmpdir=None):
    x = np.asarray(x, dtype=np.float32)
    assert x.shape == (2, 1, 48, 64, 128), x.shape
    if "nc" not in _CACHE:
        _CACHE["nc"] = _build_nc()
    nc = _CACHE["nc"]
    in_maps = _shard_inputs(x)
    res = run_bass_kernel_spmd(
        nc, in_maps, list(range(NCORES)), trace=_trace, tmpdir=_tmpdir
    )
    out = np.zeros((2, 256, 512), dtype=np.float32)
    for c in range(NCORES):
        b, q = divmod(c, 4)
        # core output is pixel-major [p, q, j]: reindex to [j, w' = 4p+q]
        # (host-side memory movement only).
        oo = res.results[c]["out"].reshape(128, 4, 64)
        out[b, 64 * q : 64 * (q + 1), :] = (
            oo.transpose(2, 0, 1).reshape(64, 512)
        )
    if _trace:
        return out, res
    return out


# revision 8
# speedup vs baseline: 1.5421x; 1.0405x over previous
"""Trainium2 Bass kernel for nn_Disp_61125974557155.

Computes: trilinear upsample of a cost volume [B,1,48,64,128] ->
[B,193,256,512] (align_corners=False, edge-replicated), softmin over
disparity, disparity regression -> [B,256,512].

Design (per core; 8 cores = 2 batches x 4 H'-quarters):
  - Host: edge-pad x (replicate), slice the core's H-halo shard, and stack a
    copy shifted by one h-row on partitions 50..99 (sharding/memory movement
    only, no arithmetic).
  - DVE: W-axis 4x lerp at low resolution -> xsw [100, 17, 4, 128] bf16.
  - PE: D-expansion with the H-axis lerp folded in (dup-shifted 100-row
    operand), all-bf16.  Tiles are paired (j1=2m, j2=2m+1 share the same
    xsw row): window = 3 PSUM banks [A1 | B1+B2 | A2] = [128, 1536].  The
    two 64-row B-chunks share the middle bank via a pair of accumulating
    matmuls whose stationaries are zero-padded into disjoint column halves
    -- no pad columns reach the ACT engine.
  - ACT: one exp per window (1536 cols) -> bf16 e-window.  ACT is the
    pacing engine; everything else hides under it.  A burst of dummy
    matmuls during the DMA head pre-warms the PE clock (HAM) so the PE can
    keep up from window 0.
  - PE: flip stat matmuls (e-slice stationary [K,128], rmat moving)
    accumulate (S0, S1) into persistent pixel-major PSUM half-banks; the
    packed B-bank yields both tiles' stats in one matmul (4 moving cols).
    Stats are split 8-before / 4-after each window's slot matmuls to keep
    the PE tail off the critical path.
  - DVE: out = S1 * recip(S0) per j-half (first half mid-loop, hidden);
    shipped pixel-major and reindexed to (j, w') by the host while
    unsharding (memory movement only).
"""

import numpy as np
from contextlib import ExitStack

import concourse.bass as bass
import concourse.bacc as bacc
import concourse.tile as tile
from concourse import mybir
from concourse.bass_utils import run_bass_kernel_spmd
from concourse.tile_rust import add_dep_helper

F32 = mybir.dt.float32
BF16 = mybir.dt.bfloat16

MAXDISP = 192
DP = MAXDISP + 1      # 193 disparities
ND = 192              # distinct d' rows (d'=0,1 coincide)
KD = 48               # low-res D
KP = KD + 2           # padded k' (edge-replicated)
NCORES = 8
WH = (0.625, 0.875, 0.125, 0.375)   # H lerp fracs per r = h' % 4
NROW = 17                            # h-rows in dup-packed shard (l = 0..16)
ROW_GROUPS = ((0, 1), (1, 1), (2, 2), (4, 4), (8, 4), (12, 4), (16, 1))
NPAIR = 32                           # tile pairs (j1=2m, j2=2m+1)
NWARM = 10                           # PE warm-up matmuls


def _build_ad() -> np.ndarray:
    """A_D [192, 50]: head-dedup'd D-axis upsample matrix on padded k'.

    Row 0 covers d' in {0,1} (edge-replicated pair, canonicalized to weight
    1.0 on xp[1]); row i (i>=1) covers d' = i+1."""
    full = np.zeros((DP, KP), dtype=np.float64)
    for dp in range(DP):
        i = (dp + 0.5) * KD / DP - 0.5
        fl = int(np.floor(i))
        fr = i - fl
        full[dp, fl + 1] += 1.0 - fr
        full[dp, fl + 2] += fr
    for dp in (0, 1):
        assert abs(full[dp, 0] + full[dp, 1] - 1.0) < 1e-12 and full[dp, 2:].sum() == 0
        full[dp, 0], full[dp, 1] = 0.0, 1.0
    return full[1:]


def _build_consts():
    ad = _build_ad()                                       # [192, 50]
    ad_a = ad[0:128]                                       # d' {0,1},2..128
    ad_b = ad[128:192]                                     # d' 129..192
    # amat [100, 8, 128], slab order = first-use order: the even-pair
    # stationaries [A0, Blo0, Bhi1, A1] then the odd-pair ones
    # [A2, Blo2, Bhi3, A3].  B slabs are zero-padded into the column half
    # their tile's B-rows occupy in the shared middle PSUM bank, so the pair
    # of B matmuls accumulates [B1; B2] with full-partition writes (and
    # FWL-friendly 128-col loads).
    amat = np.zeros((2 * KP, 8, 128), dtype=np.float64)

    def fill(slab, r, mat, h0):
        amat[:KP, slab, h0 : h0 + mat.shape[0]] = (1.0 - WH[r]) * mat.T
        amat[KP:, slab, h0 : h0 + mat.shape[0]] = WH[r] * mat.T

    for half, (r1, r2) in enumerate(((0, 1), (2, 3))):
        fill(4 * half + 0, r1, ad_a, 0)
        fill(4 * half + 1, r1, ad_b, 0)
        fill(4 * half + 2, r2, ad_b, 64)
        fill(4 * half + 3, r2, ad_a, 0)
    # rmat [128, 6]: cols 0:2 = A-chunk (S0w, S1w) for d-rows 0:128 (row0 is
    # the {0,1} pair: weights 2 and 0+1); cols 2:4 = B-stats of the pair's
    # FIRST tile (nonzero on partitions 0:64 = d' 129..192); cols 4:6 = the
    # SECOND tile's (partitions 64:128).  All entries are integers <= 256 so
    # exactly representable in bf16.
    s0w = np.ones(ND)
    s1w = np.arange(1, ND + 1, dtype=np.float64)
    s0w[0], s1w[0] = 2.0, 1.0            # row0 = {0,1}: S0w 2, S1w 0+1
    rmat = np.zeros((128, 6), dtype=np.float64)
    rmat[:, 0] = s0w[0:128]
    rmat[:, 1] = s1w[0:128]
    rmat[0:64, 2] = s0w[128:192]
    rmat[0:64, 3] = s1w[128:192]
    rmat[64:128, 4] = s0w[128:192]
    rmat[64:128, 5] = s1w[128:192]
    rbf = rmat.astype(np.float32).astype(mybir.dt.np(BF16))
    assert np.array_equal(rbf.astype(np.float64), rmat)
    return (
        np.ascontiguousarray(amat.reshape(2 * KP, 8 * 128), dtype=np.float32)
        .astype(mybir.dt.np(BF16)),
        rbf,
    )


def _build_nc() -> bass.Bass:
    nc = bacc.Bacc()
    xsd = nc.declare_dram_parameter("xsd", [2 * KP, NROW * 130], F32, isOutput=False)
    amat = nc.declare_dram_parameter("amat", [2 * KP, 8 * 128], BF16, isOutput=False)
    rmat = nc.declare_dram_parameter("rmat", [128, 6], BF16, isOutput=False)
    outp = nc.declare_dram_parameter("out", [128, 256], F32, isOutput=True)

    xsd_v = xsd.rearrange("p (h w) -> p h w", h=NROW)
    amat_v = amat.rearrange("p (v d) -> p v d", v=8)
    outp_v = outp.rearrange("p (h q j) -> p h q j", h=2, q=4)

    mult = mybir.AluOpType.mult
    add = mybir.AluOpType.add
    exp_fn = mybir.ActivationFunctionType.Exp

    # Chain PE matmuls in emission order so the scheduler keeps them
    # back-to-back (p-state / HAM ramp) and same-dtype runs contiguous.
    last_pe = [None]

    def pe_matmul(*args, **kwargs):
        ins = nc.tensor.matmul(*args, **kwargs)
        if last_pe[0] is not None:
            add_dep_helper(ins.ins, last_pe[0].ins, False,
                           "keep matmul bursts contiguous")
        last_pe[0] = ins
        return ins

    with ExitStack() as ctx:
        tc = ctx.enter_context(tile.TileContext(nc))
        singles = ctx.enter_context(tc.tile_pool(name="singles", bufs=1))
        tmp_pool = ctx.enter_context(tc.tile_pool(name="tmp", bufs=4))
        epool = ctx.enter_context(tc.tile_pool(name="epool", bufs=4))
        fin = ctx.enter_context(tc.tile_pool(name="fin", bufs=1))
        pvol = ctx.enter_context(tc.tile_pool(name="pvol", bufs=2, space="PSUM"))
        pstat = ctx.enter_context(tc.tile_pool(name="pstat", bufs=1, space="PSUM"))
        pwup = ctx.enter_context(tc.tile_pool(name="pwup", bufs=1, space="PSUM"))

        # ---- PE warm-up ----
        # The HAM clock gate only lifts the PE to 2.4 GHz after ~3.4us of
        # sustained activity, and a cold (1.2 GHz) PE cannot keep up with the
        # ACT-paced main loop -- the resulting dependency stalls then keep it
        # cold forever.  Burn the otherwise-idle DMA head on a burst of dummy
        # matmuls (zeros into the spare 8th PSUM bank) so the PE enters the
        # loop already warm.
        s_wup = singles.tile([128, 640], BF16, tag="wup")
        nc.gpsimd.memset(s_wup, 0.0)

        # ---- input loads ----
        # xsd row groups go on the sync HWDGE queue (lands earlier than the
        # gpsimd SWDGE path here), smallest/first-needed group first; the
        # stationaries go on gpsimd in two first-use-ordered chunks so both
        # queues run in parallel and nothing serializes behind 10 tiny DMAs.
        s_xsd = []
        for g, (g0, gn) in enumerate(ROW_GROUPS):
            t_x = singles.tile([2 * KP, gn, 130], F32, tag=f"xsd{g}")
            nc.sync.dma_start(out=t_x, in_=xsd_v[:, g0 : g0 + gn, :])
            s_xsd.append(t_x)
        s_am = {}
        amkeys = ((("A", 0), ("B", 0), ("B", 1), ("A", 1)),
                  (("A", 2), ("B", 2), ("B", 3), ("A", 3)))
        for half in range(2):
            t_a = singles.tile([2 * KP, 4, 128], BF16, tag=f"am{half}")
            nc.gpsimd.dma_start(
                out=t_a, in_=amat_v[:, 4 * half : 4 * half + 4, :]
            )
            for v, key in enumerate(amkeys[half]):
                s_am[key] = t_a[:, v, :]
        s_rm = singles.tile([128, 6], BF16, tag="rm")
        nc.gpsimd.dma_start(out=s_rm, in_=rmat[:, :])

        wb = pwup.tile([128, 512], F32, tag="wb")
        for _ in range(NWARM):
            pe_matmul(wb, s_wup[:, 0:128], s_wup[:, 128:640],
                      start=True, stop=True, skip_group_check=True)

        # ---- W-axis 4x lerp at low res, rw-major planes (bf16 out) ----
        s_xsw = []
        for g, (g0, gn) in enumerate(ROW_GROUPS):
            t_w = singles.tile([2 * KP, gn, 4, 128], BF16, tag=f"xsw{g}")
            t_d = tmp_pool.tile([2 * KP, gn, 129], F32, tag="wld")
            nc.vector.tensor_sub(
                t_d, s_xsd[g][:, :, 0:129], s_xsd[g][:, :, 1:130]
            )
            for rw, (coef, dc, hc) in enumerate(
                ((0.375, 0, 1), (0.125, 0, 1), (0.875, 1, 2), (0.625, 1, 2))
            ):
                nc.vector.scalar_tensor_tensor(
                    out=t_w[:, :, rw, :],
                    in0=t_d[:, :, dc : dc + 128],
                    scalar=coef,
                    in1=s_xsd[g][:, :, hc : hc + 128],
                    op0=mult,
                    op1=add,
                )
            s_xsw.append(t_w)

        def xsw_row(l: int) -> bass.AP:
            for g, (g0, gn) in enumerate(ROW_GROUPS):
                if g0 <= l < g0 + gn:
                    return s_xsw[g][:, l - g0, :, :]
            raise IndexError(l)

        # ---- persistent pixel-major stats bank, j-half-major ----
        # ps[p, h, q, j2, s] = (S0, S1) of pixel (h'-row 32h+j2, w' = 4p+q);
        # half-major so the first j-half can be finalized mid-loop.
        ps = pstat.tile([128, 512], F32, tag="ps")
        ps_h = ps.rearrange("p (h c) -> p h c", h=2)
        ps_v = [ps_h[:, h, :].rearrange("p (q j s) -> p q j s", q=4, s=2)
                for h in range(2)]
        ps_v4 = [ps_h[:, h, :].rearrange("p (q m s4) -> p q m s4", q=4, s4=4)
                 for h in range(2)]

        # ---- main loop: 32 tile pairs, one 3-bank window each ----
        # Pair m: j1 = 2m (r1 in {0,2}), j2 = 2m+1 (r2 = r1+1); both share
        # xsw row l = m//2 + m%2.
        ewins = {}

        def pair_info(m):
            t, odd = divmod(m, 2)
            r1 = 2 * odd
            return t + odd, r1, r1 + 1

        def emit_slots(m, pv):
            l, r1, r2 = pair_info(m)
            rhs = xsw_row(l).rearrange("p q s -> p (q s)")   # [100, 512]
            # Window 0's A1 bank goes to the (now idle) warm-up bank so its
            # exp only waits for this one matmul, not all four.
            a1dst = wb if m == 0 else pv[:, 0:512]
            pe_matmul(a1dst, s_am[("A", r1)], rhs, start=True, stop=True,
                      skip_group_check=True)
            # B1 (cols 0:64 live) then B2 (cols 64:128 live) accumulate into
            # the shared middle bank; the zero column-halves keep every write
            # full-partition.
            pe_matmul(pv[:, 512:1024], s_am[("B", r1)], rhs,
                      start=True, stop=False, skip_group_check=True)
            pe_matmul(pv[:, 512:1024], s_am[("B", r2)], rhs,
                      start=False, stop=True, skip_group_check=True)
            pe_matmul(pv[:, 1024:1536], s_am[("A", r2)], rhs,
                      start=True, stop=True, skip_group_check=True)

        def emit_stats(m, part):
            """part "pre" = A1 + B stats (8 mms), "post" = A2 stats (4)."""
            et = ewins[m]
            h, mm = divmod(m, 16)
            jj = 2 * mm
            first = m == 0 or m == 16
            last = m == 15 or m == NPAIR - 1
            for q in range(4):
                if part == "pre":
                    pe_matmul(
                        ps_v[h][:, q, jj, :],
                        et[:, 128 * q : 128 * q + 128],
                        s_rm[:, 0:2],
                        start=(first and q == 0), stop=False,
                        skip_group_check=True,
                    )
                    pe_matmul(
                        ps_v4[h][:, q, mm, :],
                        et[:, 512 + 128 * q : 512 + 128 * q + 128],
                        s_rm[:, 2:6],
                        start=False, stop=False,
                        skip_group_check=True,
                    )
                else:
                    pe_matmul(
                        ps_v[h][:, q, jj + 1, :],
                        et[:, 1024 + 128 * q : 1024 + 128 * q + 128],
                        s_rm[:, 0:2],
                        start=False, stop=(last and q == 3),
                        skip_group_check=True,
                    )

        def emit_finalize(h):
            rec = fin.tile([128, 4, 32], F32, tag=f"rec{h}")
            oo = fin.tile([128, 4, 32], F32, tag=f"oo{h}")
            nc.vector.reciprocal_approx_fast(rec, ps_v[h][:, :, :, 0])
            nc.vector.tensor_mul(oo, ps_v[h][:, :, :, 1], rec)
            nc.sync.dma_start(out=outp_v[:, h], in_=oo)

        for m in range(NPAIR):
            pv = pvol.tile([128, 1536], F32, tag="pv")
            et = epool.tile([128, 1536], BF16, tag="e")
            ewins[m] = et
            if m == 18:
                # stats for the first j-half (pairs 0..15) are complete;
                # finalize and ship it while the loop runs.
                emit_finalize(0)
            # stats lag two windows: exp(m-2) is complete by the time
            # slots(m) could start (its pvol buffer was recycled).  The A2
            # stats go after the slots to shorten the PE chain between
            # exp(m-2) completion and exp(m) readiness.
            if m >= 2:
                emit_stats(m - 2, "pre")
            emit_slots(m, pv)
            if m >= 2:
                emit_stats(m - 2, "post")
            if m == 0:
                nc.scalar.activation(et[:, 0:512], wb, exp_fn, scale=-1.0)
                nc.scalar.activation(et[:, 512:1536], pv[:, 512:1536],
                                     exp_fn, scale=-1.0)
            else:
                nc.scalar.activation(et, pv, exp_fn, scale=-1.0)
        for m in (NPAIR - 2, NPAIR - 1):
            emit_stats(m, "pre")
            emit_stats(m, "post")
        emit_finalize(1)

    nc.compile()
    return nc


_CACHE: dict = {}


def _shard_inputs(x: np.ndarray):
    """Edge-pad and slice per-core shards (memory movement only)."""
    xpad = np.pad(x[:, 0], ((0, 0), (1, 1), (1, 3), (1, 1)), mode="edge")
    amat, rmat = _build_consts()
    in_maps = []
    for c in range(NCORES):
        b, q = divmod(c, 4)
        xs = xpad[b][:, 16 * q : 16 * q + 18, :]          # [50, 18, 130]
        xsd = np.concatenate([xs[:, 0:17, :], xs[:, 1:18, :]], axis=0)
        xsd = np.ascontiguousarray(
            xsd.reshape(2 * KP, NROW * 130), dtype=np.float32
        )
        in_maps.append({"xsd": xsd, "amat": amat, "rmat": rmat})
    return in_maps


def kernel(x: np.ndarray, _trace: bool = False, _tmpdir=None):
    x = np.asarray(x, dtype=np.float32)
    assert x.shape == (2, 1, 48, 64, 128), x.shape
    if "nc" not in _CACHE:
        _CACHE["nc"] = _build_nc()
    nc = _CACHE["nc"]
    in_maps = _shard_inputs(x)
    res = run_bass_kernel_spmd(
        nc, in_maps, list(range(NCORES)), trace=_trace, tmpdir=_tmpdir
    )
    out = np.zeros((2, 256, 512), dtype=np.float32)
    for c in range(NCORES):
        b, q = divmod(c, 4)
        # core output is pixel-major [p, jhalf, q, j2]: reindex to
        # [j = 32h+j2, w' = 4p+q] (host-side memory movement only).
        oo = res.results[c]["out"].reshape(128, 2, 4, 32)
        out[b, 64 * q : 64 * (q + 1), :] = (
            oo.transpose(1, 3, 0, 2).reshape(64, 512)
        )
    if _trace:
        return out, res
    return out


# revision 12
# speedup vs baseline: 1.5654x; 1.0151x over previous
"""Trainium2 Bass kernel for nn_Disp_61125974557155.

Computes: trilinear upsample of a cost volume [B,1,48,64,128] ->
[B,193,256,512] (align_corners=False, edge-replicated), softmin over
disparity, disparity regression -> [B,256,512].

Design (per core; 8 cores = 2 batches x 4 H'-quarters):
  - Host: edge-pad x (replicate), slice the core's H-halo shard, and stack a
    copy shifted by one h-row on partitions 50..99 (sharding/memory movement
    only, no arithmetic).
  - DVE: W-axis 4x lerp at low resolution -> xsw [100, 17, 4, 128] bf16.
  - PE: D-expansion with the H-axis lerp folded in (dup-shifted 100-row
    operand), all-bf16.  Tiles are paired (j1=2m, j2=2m+1 share the same
    xsw row): window = 3 PSUM banks [A1 | B1+B2 | A2] = [128, 1536].  The
    two 64-row B-chunks share the middle bank via a pair of accumulating
    matmuls whose stationaries are zero-padded into disjoint column halves
    -- no pad columns reach the ACT engine.
  - ACT: one exp per window (1536 cols) -> bf16 e-window.  ACT is the
    pacing engine; everything else hides under it.  A burst of dummy
    matmuls during the DMA head pre-warms the PE clock (HAM) so the PE can
    keep up from window 0.
  - PE: flip stat matmuls (e-slice stationary [K,128], rmat moving)
    accumulate (S0, S1) into persistent pixel-major PSUM half-banks; the
    packed B-bank yields both tiles' stats in one matmul (4 moving cols).
    Stats are split 8-before / 4-after each window's slot matmuls to keep
    the PE tail off the critical path.
  - DVE: out = S1 * recip(S0) per j-half (first half mid-loop, hidden);
    shipped pixel-major and reindexed to (j, w') by the host while
    unsharding (memory movement only).
"""

import numpy as np
from contextlib import ExitStack

import concourse.bass as bass
import concourse.bacc as bacc
import concourse.tile as tile
from concourse import mybir
from concourse.bass_utils import run_bass_kernel_spmd
from concourse.tile_rust import add_dep_helper

F32 = mybir.dt.float32
BF16 = mybir.dt.bfloat16

MAXDISP = 192
DP = MAXDISP + 1      # 193 disparities
ND = 192              # distinct d' rows (d'=0,1 coincide)
KD = 48               # low-res D
KP = KD + 2           # padded k' (edge-replicated)
NCORES = 8
WH = (0.625, 0.875, 0.125, 0.375)   # H lerp fracs per r = h' % 4
NROW = 17                            # h-rows in dup-packed shard (l = 0..16)
ROW_GROUPS = ((0, 1), (1, 1), (2, 2), (4, 4), (8, 4), (12, 4), (16, 1))
NPAIR = 32                           # tile pairs (j1=2m, j2=2m+1)
NWARM = 10                           # PE warm-up matmuls (~3.4us = HAM ramp)


def _build_ad() -> np.ndarray:
    """A_D [192, 50]: head-dedup'd D-axis upsample matrix on padded k'.

    Row 0 covers d' in {0,1} (edge-replicated pair, canonicalized to weight
    1.0 on xp[1]); row i (i>=1) covers d' = i+1."""
    full = np.zeros((DP, KP), dtype=np.float64)
    for dp in range(DP):
        i = (dp + 0.5) * KD / DP - 0.5
        fl = int(np.floor(i))
        fr = i - fl
        full[dp, fl + 1] += 1.0 - fr
        full[dp, fl + 2] += fr
    for dp in (0, 1):
        assert abs(full[dp, 0] + full[dp, 1] - 1.0) < 1e-12 and full[dp, 2:].sum() == 0
        full[dp, 0], full[dp, 1] = 0.0, 1.0
    return full[1:]


def _build_consts():
    ad = _build_ad()                                       # [192, 50]
    ad_a = ad[0:128]                                       # d' {0,1},2..128
    ad_b = ad[128:192]                                     # d' 129..192
    # amat [100, 8, 128], slab order = first-use order: the even-pair
    # stationaries [A0, Blo0, Bhi1, A1] then the odd-pair ones
    # [A2, Blo2, Bhi3, A3].  B slabs are zero-padded into the column half
    # their tile's B-rows occupy in the shared middle PSUM bank, so the pair
    # of B matmuls accumulates [B1; B2] with full-partition writes (and
    # FWL-friendly 128-col loads).
    amat = np.zeros((2 * KP, 8, 128), dtype=np.float64)

    def fill(slab, r, mat, h0):
        amat[:KP, slab, h0 : h0 + mat.shape[0]] = (1.0 - WH[r]) * mat.T
        amat[KP:, slab, h0 : h0 + mat.shape[0]] = WH[r] * mat.T

    for half, (r1, r2) in enumerate(((0, 1), (2, 3))):
        fill(4 * half + 0, r1, ad_a, 0)
        fill(4 * half + 1, r1, ad_b, 0)
        fill(4 * half + 2, r2, ad_b, 64)
        fill(4 * half + 3, r2, ad_a, 0)
    # rmat [128, 6]: cols 0:2 = A-chunk (S0w, S1w) for d-rows 0:128 (row0 is
    # the {0,1} pair: weights 2 and 0+1); cols 2:4 = B-stats of the pair's
    # FIRST tile (nonzero on partitions 0:64 = d' 129..192); cols 4:6 = the
    # SECOND tile's (partitions 64:128).  All entries are integers <= 256 so
    # exactly representable in bf16.
    s0w = np.ones(ND)
    s1w = np.arange(1, ND + 1, dtype=np.float64)
    s0w[0], s1w[0] = 2.0, 1.0            # row0 = {0,1}: S0w 2, S1w 0+1
    rmat = np.zeros((128, 6), dtype=np.float64)
    rmat[:, 0] = s0w[0:128]
    rmat[:, 1] = s1w[0:128]
    rmat[0:64, 2] = s0w[128:192]
    rmat[0:64, 3] = s1w[128:192]
    rmat[64:128, 4] = s0w[128:192]
    rmat[64:128, 5] = s1w[128:192]
    rbf = rmat.astype(np.float32).astype(mybir.dt.np(BF16))
    assert np.array_equal(rbf.astype(np.float64), rmat)
    return (
        np.ascontiguousarray(amat.reshape(2 * KP, 8 * 128), dtype=np.float32)
        .astype(mybir.dt.np(BF16)),
        rbf,
    )


def _build_nc() -> bass.Bass:
    nc = bacc.Bacc()
    xsd = nc.declare_dram_parameter("xsd", [2 * KP, NROW * 130], F32, isOutput=False)
    amat = nc.declare_dram_parameter("amat", [2 * KP, 8 * 128], BF16, isOutput=False)
    rmat = nc.declare_dram_parameter("rmat", [128, 6], BF16, isOutput=False)
    outp = nc.declare_dram_parameter("out", [128, 256], F32, isOutput=True)

    xsd_v = xsd.rearrange("p (h w) -> p h w", h=NROW)
    amat_v = amat.rearrange("p (v d) -> p v d", v=8)
    outp_v = outp.rearrange("p (h q j) -> p h q j", h=2, q=4)

    mult = mybir.AluOpType.mult
    add = mybir.AluOpType.add
    exp_fn = mybir.ActivationFunctionType.Exp

    # Chain PE matmuls in emission order so the scheduler keeps them
    # back-to-back (p-state / HAM ramp) and same-dtype runs contiguous.
    last_pe = [None]

    def pe_matmul(*args, **kwargs):
        ins = nc.tensor.matmul(*args, **kwargs)
        if last_pe[0] is not None:
            add_dep_helper(ins.ins, last_pe[0].ins, False,
                           "keep matmul bursts contiguous")
        last_pe[0] = ins
        return ins

    with ExitStack() as ctx:
        tc = ctx.enter_context(tile.TileContext(nc))
        singles = ctx.enter_context(tc.tile_pool(name="singles", bufs=1))
        tmp_pool = ctx.enter_context(tc.tile_pool(name="tmp", bufs=4))
        epool = ctx.enter_context(tc.tile_pool(name="epool", bufs=4))
        fin = ctx.enter_context(tc.tile_pool(name="fin", bufs=1))
        pvol = ctx.enter_context(tc.tile_pool(name="pvol", bufs=2, space="PSUM"))
        pstat = ctx.enter_context(tc.tile_pool(name="pstat", bufs=1, space="PSUM"))
        pwup = ctx.enter_context(tc.tile_pool(name="pwup", bufs=1, space="PSUM"))

        # ---- PE warm-up ----
        # The HAM clock gate only lifts the PE to 2.4 GHz after ~3.4us of
        # sustained activity, and a cold (1.2 GHz) PE cannot keep up with the
        # ACT-paced main loop -- the resulting dependency stalls then keep it
        # cold forever.  Burn the otherwise-idle DMA head on a burst of dummy
        # matmuls (zeros into the spare 8th PSUM bank) so the PE enters the
        # loop already warm.
        s_wup = singles.tile([128, 384], BF16, tag="wup")
        nc.gpsimd.memset(s_wup, 0.0)

        # ---- input loads ----
        # xsd row groups go on the sync HWDGE queue (lands earlier than the
        # gpsimd SWDGE path here), smallest/first-needed group first; the
        # stationaries go on gpsimd in two first-use-ordered chunks so both
        # queues run in parallel and nothing serializes behind 10 tiny DMAs.
        s_xsd = []
        for g, (g0, gn) in enumerate(ROW_GROUPS):
            t_x = singles.tile([2 * KP, gn, 130], F32, tag=f"xsd{g}")
            nc.sync.dma_start(out=t_x, in_=xsd_v[:, g0 : g0 + gn, :])
            s_xsd.append(t_x)
        s_am = {}
        amkeys = ((("A", 0), ("B", 0), ("B", 1), ("A", 1)),
                  (("A", 2), ("B", 2), ("B", 3), ("A", 3)))
        for half in range(2):
            t_a = singles.tile([2 * KP, 4, 128], BF16, tag=f"am{half}")
            nc.gpsimd.dma_start(
                out=t_a, in_=amat_v[:, 4 * half : 4 * half + 4, :]
            )
            for v, key in enumerate(amkeys[half]):
                s_am[key] = t_a[:, v, :]
        s_rm = singles.tile([128, 6], BF16, tag="rm")
        nc.gpsimd.dma_start(out=s_rm, in_=rmat[:, :])

        wb = pwup.tile([128, 512], F32, tag="wb")
        for _ in range(NWARM):
            pe_matmul(wb[:, 0:384], s_wup[:, 0:128], s_wup[:, 0:384],
                      start=True, stop=True, skip_group_check=True)

        # ---- W-axis 4x lerp at low res, rw-major planes (bf16 out) ----
        s_xsw = []
        for g, (g0, gn) in enumerate(ROW_GROUPS):
            t_w = singles.tile([2 * KP, gn, 4, 128], BF16, tag=f"xsw{g}")
            t_d = tmp_pool.tile([2 * KP, gn, 129], F32, tag="wld")
            nc.vector.tensor_sub(
                t_d, s_xsd[g][:, :, 0:129], s_xsd[g][:, :, 1:130]
            )
            for rw, (coef, dc, hc) in enumerate(
                ((0.375, 0, 1), (0.125, 0, 1), (0.875, 1, 2), (0.625, 1, 2))
            ):
                nc.vector.scalar_tensor_tensor(
                    out=t_w[:, :, rw, :],
                    in0=t_d[:, :, dc : dc + 128],
                    scalar=coef,
                    in1=s_xsd[g][:, :, hc : hc + 128],
                    op0=mult,
                    op1=add,
                )
            s_xsw.append(t_w)

        def xsw_row(l: int) -> bass.AP:
            for g, (g0, gn) in enumerate(ROW_GROUPS):
                if g0 <= l < g0 + gn:
                    return s_xsw[g][:, l - g0, :, :]
            raise IndexError(l)

        # ---- persistent pixel-major stats bank, j-half-major ----
        # ps[p, h, q, j2, s] = (S0, S1) of pixel (h'-row 32h+j2, w' = 4p+q);
        # half-major so the first j-half can be finalized mid-loop.
        ps = pstat.tile([128, 512], F32, tag="ps")
        ps_h = ps.rearrange("p (h c) -> p h c", h=2)
        ps_v = [ps_h[:, h, :].rearrange("p (q j s) -> p q j s", q=4, s=2)
                for h in range(2)]
        ps_v4 = [ps_h[:, h, :].rearrange("p (q m s4) -> p q m s4", q=4, s4=4)
                 for h in range(2)]

        # ---- main loop: 32 tile pairs, one 3-bank window each ----
        # Pair m: j1 = 2m (r1 in {0,2}), j2 = 2m+1 (r2 = r1+1); both share
        # xsw row l = m//2 + m%2.
        ewins = {}

        def pair_info(m):
            t, odd = divmod(m, 2)
            r1 = 2 * odd
            return t + odd, r1, r1 + 1

        def emit_slots(m, pv):
            l, r1, r2 = pair_info(m)
            rhs = xsw_row(l).rearrange("p q s -> p (q s)")   # [100, 512]
            # Window 0's A1 bank goes to the (now idle) warm-up bank so its
            # exp only waits for this one matmul, not all four.
            a1dst = wb if m == 0 else pv[:, 0:512]
            pe_matmul(a1dst, s_am[("A", r1)], rhs, start=True, stop=True,
                      skip_group_check=True)
            # B1 (cols 0:64 live) then B2 (cols 64:128 live) accumulate into
            # the shared middle bank; the zero column-halves keep every write
            # full-partition.
            pe_matmul(pv[:, 512:1024], s_am[("B", r1)], rhs,
                      start=True, stop=False, skip_group_check=True)
            pe_matmul(pv[:, 512:1024], s_am[("B", r2)], rhs,
                      start=False, stop=True, skip_group_check=True)
            pe_matmul(pv[:, 1024:1536], s_am[("A", r2)], rhs,
                      start=True, stop=True, skip_group_check=True)

        def emit_stats(m, part):
            """part "pre" = A1 stats (4 mms), "post" = B + A2 stats (8)."""
            et = ewins[m]
            h, mm = divmod(m, 16)
            jj = 2 * mm
            first = m == 0 or m == 16
            last = m == 15 or m == NPAIR - 1
            for q in range(4):
                if part == "pre":
                    pe_matmul(
                        ps_v[h][:, q, jj, :],
                        et[:, 128 * q : 128 * q + 128],
                        s_rm[:, 0:2],
                        start=(first and q == 0), stop=False,
                        skip_group_check=True,
                    )
                else:
                    pe_matmul(
                        ps_v4[h][:, q, mm, :],
                        et[:, 512 + 128 * q : 512 + 128 * q + 128],
                        s_rm[:, 2:6],
                        start=False, stop=False,
                        skip_group_check=True,
                    )
                    pe_matmul(
                        ps_v[h][:, q, jj + 1, :],
                        et[:, 1024 + 128 * q : 1024 + 128 * q + 128],
                        s_rm[:, 0:2],
                        start=False, stop=(last and q == 3),
                        skip_group_check=True,
                    )

        def emit_finalize(h):
            rec = fin.tile([128, 4, 32], F32, tag=f"rec{h}")
            oo = fin.tile([128, 4, 32], F32, tag=f"oo{h}")
            nc.vector.reciprocal_approx_fast(rec, ps_v[h][:, :, :, 0])
            nc.vector.tensor_mul(oo, ps_v[h][:, :, :, 1], rec)
            nc.sync.dma_start(out=outp_v[:, h], in_=oo)

        for m in range(NPAIR):
            pv = pvol.tile([128, 1536], F32, tag="pv")
            et = epool.tile([128, 1536], BF16, tag="e")
            ewins[m] = et
            if m == 18:
                # stats for the first j-half (pairs 0..15) are complete;
                # finalize and ship it while the loop runs.
                emit_finalize(0)
            # stats lag two windows: exp(m-2) is complete by the time
            # slots(m) could start (its pvol buffer was recycled).  The A2
            # stats go after the slots to shorten the PE chain between
            # exp(m-2) completion and exp(m) readiness.
            if m >= 2:
                emit_stats(m - 2, "pre")
            emit_slots(m, pv)
            if m >= 2:
                emit_stats(m - 2, "post")
            if m == 0:
                nc.scalar.activation(et[:, 0:512], wb, exp_fn, scale=-1.0)
                nc.scalar.activation(et[:, 512:1536], pv[:, 512:1536],
                                     exp_fn, scale=-1.0)
            else:
                nc.scalar.activation(et, pv, exp_fn, scale=-1.0)
        for m in (NPAIR - 2, NPAIR - 1):
            emit_stats(m, "pre")
            emit_stats(m, "post")
        emit_finalize(1)

    nc.compile()
    return nc


_CACHE: dict = {}


def _shard_inputs(x: np.ndarray):
    """Edge-pad and slice per-core shards (memory movement only)."""
    xpad = np.pad(x[:, 0], ((0, 0), (1, 1), (1, 3), (1, 1)), mode="edge")
    amat, rmat = _build_consts()
    in_maps = []
    for c in range(NCORES):
        b, q = divmod(c, 4)
        xs = xpad[b][:, 16 * q : 16 * q + 18, :]          # [50, 18, 130]
        xsd = np.concatenate([xs[:, 0:17, :], xs[:, 1:18, :]], axis=0)
        xsd = np.ascontiguousarray(
            xsd.reshape(2 * KP, NROW * 130), dtype=np.float32
        )
        in_maps.append({"xsd": xsd, "amat": amat, "rmat": rmat})
    return in_maps


def kernel(x: np.ndarray, _trace: bool = False, _tmpdir=None):
    x = np.asarray(x, dtype=np.float32)
    assert x.shape == (2, 1, 48, 64, 128), x.shape
    if "nc" not in _CACHE:
        _CACHE["nc"] = _build_nc()
    nc = _CACHE["nc"]
    in_maps = _shard_inputs(x)
    res = run_bass_kernel_spmd(
        nc, in_maps, list(range(NCORES)), trace=_trace, tmpdir=_tmpdir
    )
    out = np.zeros((2, 256, 512), dtype=np.float32)
    for c in range(NCORES):
        b, q = divmod(c, 4)
        # core output is pixel-major [p, jhalf, q, j2]: reindex to
        # [j = 32h+j2, w' = 4p+q] (host-side memory movement only).
        oo = res.results[c]["out"].reshape(128, 2, 4, 32)
        out[b, 64 * q : 64 * (q + 1), :] = (
            oo.transpose(1, 3, 0, 2).reshape(64, 512)
        )
    if _trace:
        return out, res
    return out
